# revision 13
# baseline (speedup 1.0000x reference)
"""Trainium2 Bass kernel: column-parallel linear  out = input_ @ weight.T + bias.

Problem shapes (hardcoded):
    input_: [4096, 2, 4096] f32  (S, B, H)
    weight: [16384, 4096]   f32  (F, H)
    bias:   [16384]         f32
    out:    [4096, 2, 16384] f32

Tensor-parallel over the output dim F: each of the 8 cores gets the full input
and a 2048-row slice of the weight, computing its output slice locally (no
collectives needed since the host already has the full input). The host
pre-permutes both operands into the exact SBUF tile layouts (contraction dim H
on partitions, large contiguous runs per partition) and casts them to fp16, so
every DMA is a big contiguous burst; it concatenates the 8 shards at the end.

Device kernel per core: out[m, f] = sum_h XT[h, m] * WT[h, f] + bias[f]
  - fp16 operands (e5m10; inputs ~N(0,1), weights ~N(0,1/64) are well within
    range), PE upconverts to FP22 and accumulates in fp32 -> rel err ~2.5e-4
  - lhsT (stationary) = XT tile [128h, 128m], rhs (moving) = WT [128h, 512f]
  - fp16 weight loads use FWL and hide fully: ~216ns per 512-col matmul
    (8192 matmuls, ~97% of the 78.6 TFLOP/s roofline)
  - W shard fully resident in SBUF as 32 per-kt tiles [128, 2048] (fp16)
  - x-loads on SP HWDGE ring, w-loads on Act ring, out stores via SWDGE
  - consecutive matmuls accumulate into the SAME psum bank (bank alternation
    costs ~45ns/MM); bias added during PSUM->SBUF copyback on the vector engine
"""

import os
import sys

import numpy as np

for _p in ("/opt/trn_rl_repo", "/root/.axon_site/_ro/trn_rl_repo"):
    if os.path.isdir(_p) and _p not in sys.path:
        sys.path.insert(0, _p)

P = 128
FCHUNK = 512  # one PSUM bank of fp32
S, B, H, F = 4096, 2, 4096, 16384
N_CORES = 8
M = S * B
FS = F // N_CORES


def build_nc(H=H, M=M, FS=FS):
    from concourse import bacc
    import concourse.mybir as mybir
    import concourse.tile as tile

    KT, MT = H // P, M // P
    FC = min(FCHUNK, FS)
    CHUNKS = FS // FC

    f32 = mybir.dt.float32
    fp16 = mybir.dt.float16

    nc = bacc.Bacc(None, target_bir_lowering=False)
    # Pre-tiled layouts (host produces these):
    #   xt[mt, p, kt*P + mi] = fp16(input[mt*P + mi, kt*P + p])
    #   wt[p, kt*FS + fj]    = fp16(weight_shard[fj, kt*P + p])
    xt = nc.declare_dram_parameter("xt", [MT, P, KT * P], fp16, isOutput=False)
    wt = nc.declare_dram_parameter("wt", [P, KT * FS], fp16, isOutput=False)
    bias = nc.declare_dram_parameter("bias", [P, FS], f32, isOutput=False)
    out = nc.declare_dram_parameter("out", [M, FS], f32, isOutput=True)

    with tile.TileContext(nc) as tc:
        with (
            tc.tile_pool(name="wpool", bufs=KT) as wpool,
            tc.tile_pool(name="xpool", bufs=3) as xpool,
            tc.tile_pool(name="opool", bufs=3) as opool,
            tc.tile_pool(name="bpool", bufs=1) as bpool,
            tc.tile_pool(name="psum", bufs=8, space="PSUM") as pspool,
        ):
            bias_sb = bpool.tile([P, FS], f32)
            nc.scalar.dma_start(out=bias_sb[:, :], in_=bias[:, :])

            w_kt = []
            for kt in range(KT):
                wk = wpool.tile([P, FS], fp16, tag="wkt")
                nc.scalar.dma_start(out=wk[:, :], in_=wt[:, kt * FS : (kt + 1) * FS])
                w_kt.append(wk)

            for mt in range(MT):
                m0 = mt * P
                x_tile = xpool.tile([P, KT * P], fp16, tag="xtile")
                nc.sync.dma_start(out=x_tile[:, :], in_=xt[mt, :, :])
                o_tile = opool.tile([P, FS], f32, tag="otile")
                for fc in range(CHUNKS):
                    ps = pspool.tile([P, FC], f32, tag="ps")
                    for kt in range(KT):
                        nc.tensor.matmul(
                            ps[:, :],
                            lhsT=x_tile[:, kt * P : (kt + 1) * P],
                            rhs=w_kt[kt][:, fc * FC : (fc + 1) * FC],
                            start=(kt == 0),
                            stop=(kt == KT - 1),
                        )
                    nc.vector.tensor_add(
                        o_tile[:, fc * FC : (fc + 1) * FC],
                        ps[:, :],
                        bias_sb[:, fc * FC : (fc + 1) * FC],
                    )
                nc.gpsimd.dma_start(out=out[m0 : m0 + P, :], in_=o_tile[:, :])
    nc.compile()
    return nc


def make_in_maps(input_, weight, bias):
    KT, MT = H // P, M // P
    X = np.asarray(input_, dtype=np.float32).reshape(M, H).astype(np.float16)
    # xt[mt, p, kt, mi] = X[mt*P+mi, kt*P+p]
    XTt = np.ascontiguousarray(
        X.reshape(MT, P, KT, P).transpose(0, 3, 2, 1).reshape(MT, P, KT * P)
    )
    W = np.asarray(weight, dtype=np.float32).astype(np.float16)
    b = np.asarray(bias, dtype=np.float32)
    in_maps = []
    for c in range(N_CORES):
        Wc = W[c * FS : (c + 1) * FS]  # [FS, H]
        # wt[p, kt*FS + fj] = Wc[fj, kt*P+p]
        WTc = np.ascontiguousarray(
            Wc.reshape(FS, KT, P).transpose(2, 1, 0).reshape(P, KT * FS)
        )
        bc = np.ascontiguousarray(
            np.broadcast_to(b[c * FS : (c + 1) * FS][None, :], (P, FS))
        )
        in_maps.append({"xt": XTt, "wt": WTc, "bias": bc})
    return in_maps


_NC_CACHE = {}


def run_spmd(input_, weight, bias, trace=False, **kw):
    from concourse.bass_utils import run_bass_kernel_spmd

    if "full" not in _NC_CACHE:
        _NC_CACHE["full"] = build_nc()
    nc = _NC_CACHE["full"]
    in_maps = make_in_maps(input_, weight, bias)
    res = run_bass_kernel_spmd(
        nc, in_maps, core_ids=list(range(N_CORES)), trace=trace, **kw
    )
    outs = [np.asarray(res.results[c]["out"]) for c in range(N_CORES)]
    full = np.concatenate(outs, axis=1).reshape(S, B, F)
    return full, res


def kernel(input_, weight, bias):
    out, _ = run_spmd(input_, weight, bias, trace=False)
    return out


# revision 15
# speedup vs baseline: 1.0020x; 1.0020x over previous
"""Trainium2 Bass kernel: column-parallel linear  out = input_ @ weight.T + bias.

Problem shapes (hardcoded):
    input_: [4096, 2, 4096] f32  (S, B, H)
    weight: [16384, 4096]   f32  (F, H)
    bias:   [16384]         f32
    out:    [4096, 2, 16384] f32

Tensor-parallel over the output dim F: each of the 8 cores gets the full input
and a 2048-row slice of the weight, computing its output slice locally (no
collectives needed since the host already has the full input). The host
pre-permutes both operands into the exact SBUF tile layouts (contraction dim H
on partitions, large contiguous runs per partition) and casts them to fp16, so
every DMA is a big contiguous burst; it concatenates the 8 shards at the end.

Device kernel per core: out[m, f] = sum_h XT[h, m] * WT[h, f] + bias[f]
  - fp16 operands (e5m10; inputs ~N(0,1), weights ~N(0,1/64) are well within
    range), PE upconverts to FP22 and accumulates in fp32 -> rel err ~2.5e-4
  - lhsT (stationary) = XT tile [128h, 128m], rhs (moving) = WT [128h, 512f]
  - fp16 weight loads use FWL and hide fully: ~216ns per 512-col matmul
    (8192 matmuls, ~97% of the 78.6 TFLOP/s roofline)
  - W shard fully resident in SBUF as 32 per-kt tiles [128, 2048] (fp16)
  - x-loads on SP HWDGE ring, w-loads on Act ring, out stores via SWDGE
  - consecutive matmuls accumulate into the SAME psum bank (bank alternation
    costs ~45ns/MM); bias added during PSUM->SBUF copyback on the vector engine
"""

import os
import sys

import numpy as np

for _p in ("/opt/trn_rl_repo", "/root/.axon_site/_ro/trn_rl_repo"):
    if os.path.isdir(_p) and _p not in sys.path:
        sys.path.insert(0, _p)

P = 128
FCHUNK = 512  # one PSUM bank of fp32
S, B, H, F = 4096, 2, 4096, 16384
N_CORES = 8
M = S * B
FS = F // N_CORES


def build_nc(H=H, M=M, FS=FS):
    from concourse import bacc
    import concourse.mybir as mybir
    import concourse.tile as tile

    KT, MT = H // P, M // P
    FC = min(FCHUNK, FS)
    CHUNKS = FS // FC

    f32 = mybir.dt.float32
    fp16 = mybir.dt.float16

    nc = bacc.Bacc(None, target_bir_lowering=False)
    # Pre-tiled layouts (host produces these):
    #   xt[mt, p, kt*P + mi] = fp16(input[mt*P + mi, kt*P + p])
    #   wt[p, kt*FS + fj]    = fp16(weight_shard[fj, kt*P + p])
    xt = nc.declare_dram_parameter("xt", [MT, P, KT * P], fp16, isOutput=False)
    wt = nc.declare_dram_parameter("wt", [P, KT * FS], fp16, isOutput=False)
    bias = nc.declare_dram_parameter("bias", [P, FS], f32, isOutput=False)
    out = nc.declare_dram_parameter("out", [M, FS], f32, isOutput=True)

    with tile.TileContext(nc) as tc:
        with (
            tc.tile_pool(name="wpool", bufs=KT) as wpool,
            tc.tile_pool(name="xpool", bufs=3) as xpool,
            tc.tile_pool(name="opool", bufs=3) as opool,
            tc.tile_pool(name="bpool", bufs=1) as bpool,
            tc.tile_pool(name="psum", bufs=8, space="PSUM") as pspool,
        ):
            bias_sb = bpool.tile([P, FS], f32)
            nc.scalar.dma_start(out=bias_sb[:, :], in_=bias[:, :])

            w_kt = []
            for kt in range(KT):
                wk = wpool.tile([P, FS], fp16, tag="wkt")
                if kt < 4:
                    # fc-granular loads so the first matmuls wait on 128KB,
                    # not the whole 512KB tile
                    for fc in range(CHUNKS):
                        nc.scalar.dma_start(
                            out=wk[:, fc * FC : (fc + 1) * FC],
                            in_=wt[:, kt * FS + fc * FC : kt * FS + (fc + 1) * FC],
                        )
                else:
                    nc.scalar.dma_start(
                        out=wk[:, :], in_=wt[:, kt * FS : (kt + 1) * FS]
                    )
                w_kt.append(wk)

            for mt in range(MT):
                m0 = mt * P
                x_tile = xpool.tile([P, KT * P], fp16, tag="xtile")
                nc.sync.dma_start(out=x_tile[:, :], in_=xt[mt, :, :])
                o_tile = opool.tile([P, FS], f32, tag="otile")
                for fc in range(CHUNKS):
                    ps = pspool.tile([P, FC], f32, tag="ps")
                    for kt in range(KT):
                        nc.tensor.matmul(
                            ps[:, :],
                            lhsT=x_tile[:, kt * P : (kt + 1) * P],
                            rhs=w_kt[kt][:, fc * FC : (fc + 1) * FC],
                            start=(kt == 0),
                            stop=(kt == KT - 1),
                        )
                    nc.vector.tensor_add(
                        o_tile[:, fc * FC : (fc + 1) * FC],
                        ps[:, :],
                        bias_sb[:, fc * FC : (fc + 1) * FC],
                    )
                nc.scalar.dma_start(out=out[m0 : m0 + P, :], in_=o_tile[:, :])
    nc.compile()
    return nc


def make_in_maps(input_, weight, bias):
    KT, MT = H // P, M // P
    X = np.asarray(input_, dtype=np.float32).reshape(M, H).astype(np.float16)
    # xt[mt, p, kt, mi] = X[mt*P+mi, kt*P+p]
    XTt = np.ascontiguousarray(
        X.reshape(MT, P, KT, P).transpose(0, 3, 2, 1).reshape(MT, P, KT * P)
    )
    W = np.asarray(weight, dtype=np.float32).astype(np.float16)
    b = np.asarray(bias, dtype=np.float32)
    in_maps = []
    for c in range(N_CORES):
        Wc = W[c * FS : (c + 1) * FS]  # [FS, H]
        # wt[p, kt*FS + fj] = Wc[fj, kt*P+p]
        WTc = np.ascontiguousarray(
            Wc.reshape(FS, KT, P).transpose(2, 1, 0).reshape(P, KT * FS)
        )
        bc = np.ascontiguousarray(
            np.broadcast_to(b[c * FS : (c + 1) * FS][None, :], (P, FS))
        )
        in_maps.append({"xt": XTt, "wt": WTc, "bias": bc})
    return in_maps


_NC_CACHE = {}


def run_spmd(input_, weight, bias, trace=False, **kw):
    from concourse.bass_utils import run_bass_kernel_spmd

    if "full" not in _NC_CACHE:
        _NC_CACHE["full"] = build_nc()
    nc = _NC_CACHE["full"]
    in_maps = make_in_maps(input_, weight, bias)
    res = run_bass_kernel_spmd(
        nc, in_maps, core_ids=list(range(N_CORES)), trace=trace, **kw
    )
    outs = [np.asarray(res.results[c]["out"]) for c in range(N_CORES)]
    full = np.concatenate(outs, axis=1).reshape(S, B, F)
    return full, res


def kernel(input_, weight, bias):
    out, _ = run_spmd(input_, weight, bias, trace=False)
    return out
